# revision 12
# baseline (speedup 1.0000x reference)
"""Trainium2 Bass kernel for nn_CausalTransformer (B=2, S=1024, H=1024, 16 heads,
8 layers, FF=4096, rel-pos bias + causal mask, patch embed TIN=5).

Distribution over 8 NeuronCores: two groups of 4 (one per batch element).
Within a group the residual stream is sequence-sharded (256 tokens/core,
feature-major [feature_partition, token] layout); attention is head-sharded
(4 heads/core). Two 4-core AllGathers per layer (post-lnA activations and
attention output, both bf16). All matmuls run in bf16 with fp32 PSUM
accumulation; the residual stream stays fp32 in SBUF.
"""
import sys

if "/opt/trn_rl_repo" not in sys.path:
    sys.path.insert(0, "/opt/trn_rl_repo")

import numpy as np
import ml_dtypes

import concourse.bacc as bacc
import concourse.mybir as mybir
import concourse.tile as tile
from concourse.masks import make_identity
from concourse.bass_utils import run_bass_kernel_spmd

BF = mybir.dt.bfloat16
F32 = mybir.dt.float32
AF = mybir.ActivationFunctionType
OP = mybir.AluOpType

N_CORES = 8
GROUPS = [[0, 1, 2, 3], [4, 5, 6, 7]]
R = 4              # ranks per group
T = 256            # tokens per core
S = 1024           # tokens per batch
HDIM = 1024
N_HEADS = 16
HL = 4             # heads per core
HEAD_DIM = 64
FF = 4096
NLAYERS = 8
TIN = 5
FEAT = 256
IN_DIM = TIN * FEAT  # 1280
NCLS = 100
MAX_REL = 128
SCALE = 1.0 / np.sqrt(HEAD_DIM)


def _layernorm(nc, sb, lnp, ps_big, ps_rs, x, out, g_ap, b_ap, eps, nfb, csts):
    """Feature-major layernorm: x [128, nfb, T] fp32 -> out (bf16 or f32).

    Stats via bf16 cast + ones-matmul partition reduction; apply with
    per-partition g/b on the scalar engine. g_ap/b_ap: SBUF [128, nfb]."""
    ones, _, _, _ = csts
    D = nfb * 128
    xb = lnp.tile([128, nfb, T], BF, tag="ln_xb")
    nc.vector.tensor_copy(xb, x)
    x2 = lnp.tile([128, nfb, T], BF, tag="ln_x2")
    nc.vector.tensor_tensor(x2, xb, xb, OP.mult)
    st = ps_rs.tile([1, 2, T], F32, tag="rs")
    for fb in range(nfb):
        nc.tensor.matmul(st[:, 0, :], ones, xb[:, fb, :],
                         start=(fb == 0), stop=(fb == nfb - 1))
    for fb in range(nfb):
        nc.tensor.matmul(st[:, 1, :], ones, x2[:, fb, :],
                         start=(fb == 0), stop=(fb == nfb - 1))
    # m = sum/D ; var = sumsq/D - m^2 ; rstd = 1/sqrt(var+eps) ; mr = m*rstd
    mv = sb.tile([1, 2, T], F32, tag="ln_mv")       # [m, ex2]
    nc.vector.tensor_scalar_mul(mv, st, 1.0 / D)
    var = sb.tile([1, T], F32, tag="ln_var")
    nc.vector.tensor_tensor(var, mv[:, 0, :], mv[:, 0, :], OP.mult)
    nc.vector.tensor_tensor(var, mv[:, 1, :], var, OP.subtract)
    eps_t = sb.tile([1, 1], F32, tag="ln_eps")
    nc.vector.memset(eps_t, float(eps))
    sd = sb.tile([1, T], F32, tag="ln_sd")
    nc.scalar.activation(sd, var, AF.Sqrt, bias=eps_t)
    rstd = sb.tile([1, T], F32, tag="ln_rstd")
    nc.vector.reciprocal(rstd, sd)
    rmr = sb.tile([1, 2, T], BF, tag="ln_rmr")      # [rstd, m*rstd] bf16
    nc.vector.tensor_copy(rmr[:, 0, :], rstd)
    nc.vector.tensor_tensor(rmr[:, 1, :], mv[:, 0, :], rstd, OP.mult)
    # broadcast to 128 partitions via rank-1 matmul
    _, ones1, _, _ = csts
    bc = ps_big.tile([128, 512], F32, tag="mm")
    nc.tensor.matmul(bc[:, 0:2 * T], ones1, rmr[:, :, :], start=True, stop=True)
    bcs = sb.tile([128, 2, T], BF, tag="ln_bcs")
    nc.vector.tensor_copy(bcs, bc[:, 0:2 * T])
    # u = x*rstd - m*rstd ; out = u*g + b (per-partition scale/bias)
    u = lnp.tile([128, nfb, T], BF, tag="ln_u")
    nc.vector.tensor_tensor(
        u, xb, bcs[:, 0, None, :].to_broadcast([128, nfb, T]), OP.mult)
    nc.vector.tensor_tensor(
        u, u, bcs[:, 1, None, :].to_broadcast([128, nfb, T]), OP.subtract)
    for fb in range(nfb):
        nc.scalar.activation(out[:, fb, :], u[:, fb, :], AF.Identity,
                             bias=b_ap[:, fb:fb + 1], scale=g_ap[:, fb:fb + 1])


def build(nlayers=NLAYERS, debug=False, gelu_fn=None):
    if gelu_fn is None:
        gelu_fn = AF.Gelu
    nc = bacc.Bacc(num_devices=N_CORES)

    # ---- I/O declarations
    def inp(name, shape, dt):
        return nc.dram_tensor(name, shape, dt, kind="ExternalInput")

    xT = inp("xT", [IN_DIM, T], F32)
    expb = inp("expb", [HL, 4, 128, T], BF)
    qkvw = inp("qkvw", [nlayers, HDIM, 512], BF)
    vw = inp("vw", [nlayers, HDIM, 256], BF)
    qkvb = inp("qkvb", [nlayers, 512], F32)
    vb = inp("vb", [nlayers, 256], F32)
    pe_wT = inp("pe_wT", [IN_DIM, HDIM], BF)
    pe_b = inp("pe_b", [HDIM], F32)
    pe_ln1_g = inp("pe_ln1_g", [IN_DIM], F32)
    pe_ln1_b = inp("pe_ln1_b", [IN_DIM], F32)
    pe_ln2_g = inp("pe_ln2_g", [HDIM], F32)
    pe_ln2_b = inp("pe_ln2_b", [HDIM], F32)
    lnA_g = inp("lnA_g", [nlayers, HDIM], F32)
    lnA_b = inp("lnA_b", [nlayers, HDIM], F32)
    lnF_g = inp("lnF_g", [nlayers, HDIM], F32)
    lnF_b = inp("lnF_b", [nlayers, HDIM], F32)
    outw = inp("outw", [nlayers, 256, HDIM], BF)
    outb = inp("outb", [nlayers, HDIM], F32)
    fc1w = inp("fc1w", [nlayers, HDIM, FF], BF)
    fc1b = inp("fc1b", [nlayers, FF], F32)
    fc2w = inp("fc2w", [nlayers, FF, HDIM], BF)
    fc2b = inp("fc2b", [nlayers, HDIM], F32)
    fin_g = inp("fin_g", [HDIM], F32)
    fin_b = inp("fin_b", [HDIM], F32)
    headw = inp("headw", [HDIM, NCLS], BF)
    headb = inp("headb", [NCLS], F32)

    out = nc.dram_tensor("out", [T, NCLS], F32, kind="ExternalOutput")
    dbg = {}
    if debug:
        for name, shape, dt in [
                ("dbg_z0", [128, 8, T], F32), ("dbg_ha", [128, 8, T], BF),
                ("dbg_hall", [128, 8, R, T], BF),
                ("dbg_qk", [128, 4, S], BF), ("dbg_v", [128, 8, 256], BF),
                ("dbg_o", [128, 2, S], BF), ("dbg_z1", [128, 8, T], F32)]:
            dbg[name] = nc.dram_tensor(name, shape, dt, kind="ExternalOutput")

    with tile.TileContext(nc) as tc:
        pools = [
            tc.tile_pool(name="persist", bufs=1),
            tc.tile_pool(name="sb", bufs=2),
            tc.tile_pool(name="big", bufs=1),
            tc.tile_pool(name="wpool", bufs=3),
            tc.tile_pool(name="wpool2", bufs=2),
            tc.tile_pool(name="attp", bufs=3),
            tc.tile_pool(name="lnp", bufs=1),
            tc.tile_pool(name="dram", bufs=1, space="DRAM"),
            tc.tile_pool(name="ps_big", bufs=3, space="PSUM"),
            tc.tile_pool(name="ps_o", bufs=2, space="PSUM"),
            tc.tile_pool(name="ps_rs", bufs=2, space="PSUM"),
        ]
        ctxs = [p.__enter__() for p in pools]
        (persist, sb, big, wpool, wpool2, attp, lnp, dram,
         ps_big, ps_o, ps_rs) = ctxs

        # ---- constants
        ones = persist.tile([128, 1], BF)
        nc.vector.memset(ones, 1.0)
        ones1 = persist.tile([1, 128], BF)
        nc.vector.memset(ones1, 1.0)
        ind_lo = persist.tile([1, 128], BF)
        nc.vector.memset(ind_lo, 0.0)
        nc.vector.memset(ind_lo[:, 0:64], 1.0)
        ind_hi = persist.tile([1, 128], BF)
        nc.vector.memset(ind_hi, 0.0)
        nc.vector.memset(ind_hi[:, 64:128], 1.0)
        ident = persist.tile([128, 128], F32)
        make_identity(nc, ident)
        csts = (ones, ones1, ind_lo, ind_hi)

        expb_sb = persist.tile([128, HL, 4, T], BF)
        nc.sync.dma_start(out=expb_sb, in_=expb.rearrange("h i p q -> p h i q"))

        def load_gb(g_dram, b_dram, nfb, tag):
            g_sb = sb.tile([128, nfb], F32, tag=f"{tag}_g")
            b_sb = sb.tile([128, nfb], F32, tag=f"{tag}_b")
            nc.sync.dma_start(out=g_sb, in_=g_dram.rearrange("(fb p) -> p fb", p=128))
            nc.sync.dma_start(out=b_sb, in_=b_dram.rearrange("(fb p) -> p fb", p=128))
            return g_sb, b_sb

        z = persist.tile([128, 8, T], F32)

        # ---- patch embed
        x_sb = big.tile([128, 10, T], F32, tag="x_sb")
        nc.sync.dma_start(out=x_sb, in_=xT.rearrange("(fb p) t -> p fb t", p=128))
        g1, b1 = load_gb(pe_ln1_g, pe_ln1_b, 10, "peln1")
        h0 = big.tile([128, 10, T], BF, tag="h0")
        _layernorm(nc, sb, lnp, ps_big, ps_rs, x_sb, h0, g1, b1, 1e-6, 10, csts)
        peb_sb, _unused = load_gb(pe_b, pe_b, 8, "peb")
        zpre = big.tile([128, 8, T], F32, tag="zpre")
        for ob in range(8):
            pw = wpool.tile([128, 10, 128], BF, tag="wo")
            nc.sync.dma_start(
                out=pw, in_=pe_wT[:, ob * 128:(ob + 1) * 128]
                    .rearrange("(ib p) o -> p ib o", p=128))
            pp = ps_big.tile([128, 512], F32, tag="mm")
            for ib in range(10):
                nc.tensor.matmul(pp[:, 0:T], pw[:, ib, :],
                                 h0[:, ib, :], start=(ib == 0), stop=(ib == 9))
            nc.scalar.activation(zpre[:, ob, :], pp[:, 0:T], AF.Identity,
                                 bias=peb_sb[:, ob:ob + 1])
        g2, b2 = load_gb(pe_ln2_g, pe_ln2_b, 8, "peln2")
        _layernorm(nc, sb, lnp, ps_big, ps_rs, zpre, z, g2, b2, 1e-6, 8, csts)
        if debug:
            nc.sync.dma_start(out=dbg["dbg_z0"][:, :, :], in_=z)

        # ---- layers
        for l in range(nlayers):
            # lnA -> ha (bf16)
            ga, ba = load_gb(lnA_g[l], lnA_b[l], 8, "lnA")
            ha = big.tile([128, 8, T], BF, tag="hx")
            _layernorm(nc, sb, lnp, ps_big, ps_rs, z, ha, ga, ba, 1e-5, 8, csts)
            if debug and l == 0:
                nc.sync.dma_start(out=dbg["dbg_ha"][:, :, :], in_=ha)
            # AllGather h
            agh_in = dram.tile([HDIM, T], BF, tag="agh_in")
            agh_out = dram.tile([R, HDIM, T], BF, tag="agh_out")
            nc.sync.dma_start(out=agh_in.rearrange("(fb p) t -> p fb t", p=128),
                              in_=ha)
            nc.gpsimd.collective_compute(
                "AllGather", OP.bypass, replica_groups=GROUPS,
                ins=[agh_in.opt()], outs=[agh_out.opt()])
            h_all = big.tile([128, 8, R, T], BF, tag="h_all")
            for r in range(R):
                nc.sync.dma_start(
                    out=h_all[:, :, r, :],
                    in_=agh_out[r].rearrange("(fb p) t -> p fb t", p=128))
            if debug and l == 0:
                nc.sync.dma_start(out=dbg["dbg_hall"][:, :, :, :], in_=h_all)

            # qk projections: [128 if, 8 ifb, (r t)] x w -> qk_sb [128, 4 ofb, S]
            wqk = wpool2.tile([128, 8, 512], BF, tag="wqk")
            nc.sync.dma_start(out=wqk,
                              in_=qkvw[l].rearrange("(ib p) o -> p ib o", p=128))
            qkvb_sb, _uqb = load_gb(qkvb[l], qkvb[l], 4, "qkvb")
            vb_row = sb.tile([1, 256], F32, tag="vbrow")
            nc.sync.dma_start(out=vb_row, in_=vb[l][None, :])
            vb_rowb = sb.tile([1, 256], BF, tag="vbrowb")
            nc.vector.tensor_copy(vb_rowb, vb_row)
            qk_sb = big.tile([128, 4, S], BF, tag="qk_sb")
            hflat = h_all  # [128, 8, R, T] ; token chunks of 512 = 2 r-blocks
            for ob in range(4):
                for tch in range(2):
                    pp = ps_big.tile([128, 512], F32, tag="mm")
                    for ib in range(8):
                        nc.tensor.matmul(
                            pp, wqk[:, ib, ob * 128:(ob + 1) * 128],
                            hflat[:, ib, 2 * tch:2 * tch + 2, :],
                            start=(ib == 0), stop=(ib == 7))
                    nc.scalar.activation(qk_sb[:, ob, 512 * tch:512 * (tch + 1)],
                                         pp, AF.Identity, bias=qkvb_sb[:, ob:ob + 1])
            if debug and l == 0:
                nc.sync.dma_start(out=dbg["dbg_qk"][:, :, :], in_=qk_sb)

            # v projection: token-major v_sb [128 tok, 8 tb, 256 vf]
            wv = big.tile([128, 8, 256], BF, tag="wv")
            nc.sync.dma_start(out=wv,
                              in_=vw[l].rearrange("(ib p) o -> p ib o", p=128))
            v_sb = big.tile([128, 8, 256], BF, tag="v_sb")
            for tb in range(8):
                r, half = tb // 2, tb % 2
                pp = ps_big.tile([128, 512], F32, tag="mm")
                for ib in range(8):
                    nc.tensor.matmul(
                        pp[:, 0:256],
                        hflat[:, ib, r, half * 128:(half + 1) * 128],
                        wv[:, ib, :], start=(ib == 0), stop=False)
                nc.tensor.matmul(pp[:, 0:256], ones1, vb_rowb,
                                 start=False, stop=True)
                nc.vector.tensor_copy(v_sb[:, tb, :], pp[:, 0:256])
            if debug and l == 0:
                nc.sync.dma_start(out=dbg["dbg_v"][:, :, :], in_=v_sb)

            # attention, head pairs
            o_all = big.tile([128, 2, S], BF, tag="o_all")
            for hp in range(2):
                h0i, h1i = 2 * hp, 2 * hp + 1   # local head indices
                qofb, kofb = hp, 2 + hp
                for qc in range(4):
                    nk = 2 * qc + 2
                    o_ps = ps_o.tile([128, T], F32, tag="o")
                    rs = ps_rs.tile([1, 2, T], F32, tag="rs")
                    for kblk in range(nk):
                        idx = max(kblk - 2 * qc + 2, 0)
                        s0 = ps_big.tile([128, 512], F32, tag="mm")
                        nc.tensor.matmul(
                            s0[:, 0:T],
                            qk_sb[0:64, kofb, kblk * 128:(kblk + 1) * 128],
                            qk_sb[0:64, qofb, qc * T:(qc + 1) * T],
                            start=True, stop=True)
                        s1 = ps_big.tile([128, 512], F32, tag="mm")
                        nc.tensor.matmul(
                            s1[:, 0:T],
                            qk_sb[64:128, kofb, kblk * 128:(kblk + 1) * 128],
                            qk_sb[64:128, qofb, qc * T:(qc + 1) * T],
                            start=True, stop=True, tile_position=(64, 0))
                        pT = attp.tile([128, 2, T], BF, tag="pT")
                        e0 = attp.tile([128, 2, T], BF, tag="e0")
                        nc.scalar.activation(e0[:, 0, :], s0[:, 0:T], AF.Exp,
                                             scale=float(SCALE))
                        nc.scalar.activation(e0[:, 1, :], s1[:, 0:T], AF.Exp,
                                             scale=float(SCALE))
                        nc.vector.tensor_tensor(
                            pT[:, 0, :], e0[:, 0, :], expb_sb[:, h0i, idx, :],
                            OP.mult)
                        nc.vector.tensor_tensor(
                            pT[:, 1, :], e0[:, 1, :], expb_sb[:, h1i, idx, :],
                            OP.mult)
                        nc.tensor.matmul(rs, ones, pT,
                                         start=(kblk == 0), stop=(kblk == nk - 1))
                        nc.tensor.matmul(
                            o_ps[0:64, :], v_sb[:, kblk, h0i * 64:(h0i + 1) * 64],
                            pT[:, 0, :], start=(kblk == 0), stop=(kblk == nk - 1),
                            skip_group_check=True)
                        nc.tensor.matmul(
                            o_ps[64:128, :], v_sb[:, kblk, h1i * 64:(h1i + 1) * 64],
                            pT[:, 1, :], start=(kblk == 0), stop=(kblk == nk - 1),
                            tile_position=(0, 64), skip_group_check=True)
                    rc = sb.tile([1, 2, T], F32, tag="rc")
                    nc.vector.reciprocal(rc, rs)
                    rcb = sb.tile([1, 2, T], BF, tag="rcb")
                    nc.vector.tensor_copy(rcb, rc)
                    bc = ps_big.tile([128, 512], F32, tag="mm")
                    nc.tensor.matmul(bc[:, 0:T], ind_lo, rcb[:, 0, :],
                                     start=True, stop=False)
                    nc.tensor.matmul(bc[:, 0:T], ind_hi, rcb[:, 1, :],
                                     start=False, stop=True)
                    bcs = sb.tile([128, T], BF, tag="bcs")
                    nc.scalar.copy(bcs, bc[:, 0:T])
                    nc.vector.tensor_tensor(
                        o_all[:, hp, qc * T:(qc + 1) * T], o_ps, bcs, OP.mult)
            if debug and l == 0:
                nc.sync.dma_start(out=dbg["dbg_o"][:, :, :], in_=o_all)

            # out-proj partials (my head features x all tokens) -> ReduceScatter
            wo_sh = big.tile([128, 2, HDIM], BF, tag="wo_sh")
            nc.sync.dma_start(
                out=wo_sh, in_=outw[l].rearrange("(ib p) o -> p ib o", p=128))
            rs_in = dram.tile([R, HDIM, T], BF, tag="rs_in")
            rs_out = dram.tile([HDIM, T], BF, tag="rs_out")
            rs_in_v = rs_in.rearrange("r (ob p) t -> p ob r t", p=128)
            for ob in range(8):
                for tch in range(2):
                    pp = ps_big.tile([128, 512], F32, tag="mm")
                    for ib in range(2):
                        nc.tensor.matmul(
                            pp, wo_sh[:, ib, ob * 128:(ob + 1) * 128],
                            o_all[:, ib, tch * 512:(tch + 1) * 512],
                            start=(ib == 0), stop=(ib == 1))
                    zp = sb.tile([128, 2, T], BF, tag="zp")
                    nc.scalar.copy(zp, pp)
                    nc.sync.dma_start(
                        out=rs_in_v[:, ob, 2 * tch:2 * tch + 2, :], in_=zp)
            nc.gpsimd.collective_compute(
                "ReduceScatter", OP.add, replica_groups=GROUPS,
                ins=[rs_in.opt()], outs=[rs_out.opt()])
            zr = big.tile([128, 8, T], BF, tag="zr")
            nc.sync.dma_start(out=zr,
                              in_=rs_out.rearrange("(ob p) t -> p ob t", p=128))
            outb_sb, _u2 = load_gb(outb[l], outb[l], 8, "outb")
            for ob in range(8):
                nc.vector.scalar_tensor_tensor(
                    z[:, ob, :], zr[:, ob, :], outb_sb[:, ob:ob + 1], z[:, ob, :],
                    OP.add, OP.add)

            # lnF -> hf
            gf, bf_ = load_gb(lnF_g[l], lnF_b[l], 8, "lnF")
            hf = big.tile([128, 8, T], BF, tag="hx")
            _layernorm(nc, sb, lnp, ps_big, ps_rs, z, hf, gf, bf_, 1e-5, 8, csts)
            # fc1 + gelu
            f1b_sb = sb.tile([128, 32], F32, tag="f1b")
            nc.sync.dma_start(out=f1b_sb,
                              in_=fc1b[l].rearrange("(fb p) -> p fb", p=128))
            gact = big.tile([128, 32, T], BF, tag="gact")
            for ob in range(32):
                wf = wpool.tile([128, 8, 128], BF, tag="wo")
                nc.sync.dma_start(
                    out=wf,
                    in_=fc1w[l][:, ob * 128:(ob + 1) * 128]
                        .rearrange("(ib p) o -> p ib o", p=128))
                pp = ps_big.tile([128, 512], F32, tag="mm")
                for ib in range(8):
                    nc.tensor.matmul(pp[:, 0:T], wf[:, ib, :], hf[:, ib, :],
                                     start=(ib == 0), stop=(ib == 7))
                nc.scalar.activation(gact[:, ob, :], pp[:, 0:T], gelu_fn,
                                     bias=f1b_sb[:, ob:ob + 1])
            # fc2 += z
            f2b_sb, _u3 = load_gb(fc2b[l], fc2b[l], 8, "f2b")
            for ob in range(8):
                wf2 = wpool2.tile([128, 32, 128], BF, tag="wf2")
                nc.sync.dma_start(
                    out=wf2,
                    in_=fc2w[l][:, ob * 128:(ob + 1) * 128]
                        .rearrange("(ib p) o -> p ib o", p=128))
                pp = ps_big.tile([128, 512], F32, tag="mm")
                for ib in range(32):
                    nc.tensor.matmul(pp[:, 0:T], wf2[:, ib, :], gact[:, ib, :],
                                     start=(ib == 0), stop=(ib == 31))
                nc.vector.scalar_tensor_tensor(
                    z[:, ob, :], pp[:, 0:T], f2b_sb[:, ob:ob + 1], z[:, ob, :],
                    OP.add, OP.add)
            if debug and l == 0:
                nc.sync.dma_start(out=dbg["dbg_z1"][:, :, :], in_=z)

        # ---- final LN + head
        gg, bb = load_gb(fin_g, fin_b, 8, "fin")
        hfin = big.tile([128, 8, T], BF, tag="hx")
        _layernorm(nc, sb, lnp, ps_big, ps_rs, z, hfin, gg, bb, 1e-5, 8, csts)
        hw_sb = big.tile([128, 8, NCLS], BF, tag="hw")
        nc.sync.dma_start(out=hw_sb,
                          in_=headw.rearrange("(ib p) o -> p ib o", p=128))
        hb_sb = sb.tile([NCLS, 1], F32, tag="hb")
        nc.sync.dma_start(out=hb_sb, in_=headb[:, None])
        lp = ps_big.tile([128, 512], F32, tag="mm")
        for ib in range(8):
            nc.tensor.matmul(lp[0:NCLS, 0:T], hw_sb[:, ib, :], hfin[:, ib, :],
                             start=(ib == 0), stop=(ib == 7))
        logit_fm = big.tile([NCLS, T], F32, tag="logit")
        nc.scalar.activation(logit_fm, lp[0:NCLS, 0:T], AF.Identity,
                             bias=hb_sb[:, 0:1])
        # transpose to token-major and store
        osb = big.tile([128, 2, NCLS], F32, tag="osb")
        for ch in range(2):
            tp = ps_big.tile([128, 512], F32, tag="mm")
            nc.tensor.transpose(tp[:, 0:NCLS],
                                logit_fm[:, ch * 128:(ch + 1) * 128],
                                ident[0:NCLS, 0:NCLS])
            nc.vector.tensor_copy(osb[:, ch, :], tp[:, 0:NCLS])
        nc.sync.dma_start(out=out.rearrange("(c p) n -> p c n", p=128), in_=osb)

        for p in reversed(pools):
            p.__exit__(None, None, None)
    nc.compile()
    return nc


def prep_inputs(inputs, nlayers=NLAYERS):
    """Host-side: shard + transpose + cast. Returns in_maps (list of 8 dicts)."""
    bf = ml_dtypes.bfloat16
    f32 = np.float32

    x = np.asarray(inputs["x"], f32)            # (2, 5120, 256)
    qkv_w = np.asarray(inputs["qkv_w"], f32)    # (L, 3072, 1024)
    rel = np.asarray(inputs["rel_table"], f32)  # (16, 257)

    shared = {
        "pe_wT": np.ascontiguousarray(np.asarray(inputs["pe_w"], f32).T).astype(bf),
        "pe_b": np.asarray(inputs["pe_b"], f32),
        "pe_ln1_g": np.asarray(inputs["pe_ln1_g"], f32),
        "pe_ln1_b": np.asarray(inputs["pe_ln1_b"], f32),
        "pe_ln2_g": np.asarray(inputs["pe_ln2_g"], f32),
        "pe_ln2_b": np.asarray(inputs["pe_ln2_b"], f32),
        "lnA_g": np.asarray(inputs["lnA_g"], f32)[:nlayers],
        "lnA_b": np.asarray(inputs["lnA_b"], f32)[:nlayers],
        "lnF_g": np.asarray(inputs["lnF_g"], f32)[:nlayers],
        "lnF_b": np.asarray(inputs["lnF_b"], f32)[:nlayers],
        "outb": np.asarray(inputs["out_b"], f32)[:nlayers],
        "fc1w": np.ascontiguousarray(
            np.asarray(inputs["fc1_w"], f32)[:nlayers].transpose(0, 2, 1)).astype(bf),
        "fc1b": np.asarray(inputs["fc1_b"], f32)[:nlayers],
        "fc2w": np.ascontiguousarray(
            np.asarray(inputs["fc2_w"], f32)[:nlayers].transpose(0, 2, 1)).astype(bf),
        "fc2b": np.asarray(inputs["fc2_b"], f32)[:nlayers],
        "fin_g": np.asarray(inputs["final_g"], f32),
        "fin_b": np.asarray(inputs["final_b"], f32),
        "headw": np.ascontiguousarray(np.asarray(inputs["head_w"], f32).T).astype(bf),
        "headb": np.asarray(inputs["head_b"], f32),
    }

    # per-position (p in 0..3) sharded tensors
    per_pos = []
    for p in range(R):
        heads = range(HL * p, HL * p + HL)
        qrows = np.concatenate([np.arange(h * 64, h * 64 + 64) for h in heads])
        krows = HDIM + qrows
        vrows = 2 * HDIM + qrows
        sel_qk = np.concatenate([qrows, krows])
        qkvw_p = np.ascontiguousarray(
            qkv_w[:nlayers, sel_qk, :].transpose(0, 2, 1)).astype(bf)
        vw_p = np.ascontiguousarray(
            qkv_w[:nlayers, vrows, :].transpose(0, 2, 1)).astype(bf)
        qkv_b = np.asarray(inputs["qkv_b"], f32)
        qkvb_p = qkv_b[:nlayers][:, sel_qk].copy()
        vb_p = qkv_b[:nlayers][:, vrows].copy()
        # expb tiles: idx 0 = const (d<=-256); idx 1..3: d = -128, 0, 128
        expb_p = np.zeros((HL, 4, 128, T), np.float32)
        kk = np.arange(128)[:, None]
        qq = np.arange(T)[None, :]
        for hh, h in enumerate(heads):
            expb_p[hh, 0] = np.exp(rel[h, 0])
            for i, d in enumerate([-128, 0, 128]):
                diff = d + kk - qq
                val = np.exp(rel[h, np.clip(diff, -MAX_REL, MAX_REL) + MAX_REL])
                val[diff > 0] = 0.0
                expb_p[hh, 1 + i] = val
        out_w = np.asarray(inputs["out_w"], f32)
        outw_p = np.ascontiguousarray(
            out_w[:nlayers].transpose(0, 2, 1)[:, qrows, :]).astype(bf)
        per_pos.append({
            "qkvw": qkvw_p, "vw": vw_p, "qkvb": qkvb_p, "vb": vb_p,
            "expb": expb_p.astype(bf), "outw": outw_p,
        })

    in_maps = []
    for c in range(N_CORES):
        b, p = c // R, c % R
        xs = x[b, p * T * TIN:(p + 1) * T * TIN, :]        # (1280, 256)
        xTc = np.ascontiguousarray(
            xs.reshape(T, IN_DIM).T)                        # (1280, 256)
        m = {"xT": xTc.astype(f32)}
        m.update(per_pos[p])
        m.update(shared)
        in_maps.append(m)
    return in_maps


_BUILD_CACHE = {}


def kernel(**inputs) -> np.ndarray:
    in_maps = prep_inputs(inputs)
    key = NLAYERS
    if key not in _BUILD_CACHE:
        _BUILD_CACHE[key] = build(NLAYERS)
    nc = _BUILD_CACHE[key]
    res = run_bass_kernel_spmd(nc, in_maps, core_ids=list(range(N_CORES)))
    B = 2
    out = np.zeros((B, S, NCLS), np.float32)
    for c in range(N_CORES):
        b, p = c // R, c % R
        out[b, p * T:(p + 1) * T, :] = res.results[c]["out"]
    return out


# revision 14
# speedup vs baseline: 34.7208x; 34.7208x over previous
"""Trainium2 Bass kernel for nn_CausalTransformer (B=2, S=1024, H=1024, 16 heads,
8 layers, FF=4096, rel-pos bias + causal mask, patch embed TIN=5).

Distribution over 8 NeuronCores: two groups of 4 (one per batch element).
Within a group the residual stream is sequence-sharded (256 tokens/core,
feature-major [feature_partition, token] layout); attention is head-sharded
(4 heads/core). Two 4-core AllGathers per layer (post-lnA activations and
attention output, both bf16). All matmuls run in bf16 with fp32 PSUM
accumulation; the residual stream stays fp32 in SBUF.
"""
import sys

if "/opt/trn_rl_repo" not in sys.path:
    sys.path.insert(0, "/opt/trn_rl_repo")

import numpy as np
import ml_dtypes

import concourse.bacc as bacc
import concourse.mybir as mybir
import concourse.tile as tile
from concourse.masks import make_identity
from concourse.bass_utils import run_bass_kernel_spmd

BF = mybir.dt.bfloat16
F32 = mybir.dt.float32
AF = mybir.ActivationFunctionType
OP = mybir.AluOpType

N_CORES = 8
GROUPS = [[0, 1, 2, 3], [4, 5, 6, 7]]
R = 4              # ranks per group
T = 256            # tokens per core
S = 1024           # tokens per batch
HDIM = 1024
N_HEADS = 16
HL = 4             # heads per core
HEAD_DIM = 64
FF = 4096
NLAYERS = 8
TIN = 5
FEAT = 256
IN_DIM = TIN * FEAT  # 1280
NCLS = 100
MAX_REL = 128
SCALE = 1.0 / np.sqrt(HEAD_DIM)


def _layernorm(nc, sb, lnp, ps_big, ps_rs, x, out, g_ap, b_ap, eps, nfb, csts):
    """Feature-major layernorm: x [128, nfb, T] fp32 -> out (bf16 or f32).

    Stats via bf16 cast + ones-matmul partition reduction; apply with
    per-partition g/b on the scalar engine. g_ap/b_ap: SBUF [128, nfb]."""
    ones, _, _, _ = csts
    D = nfb * 128
    xb = lnp.tile([128, nfb, T], BF, tag="ln_xb")
    nc.vector.tensor_copy(xb, x)
    x2 = lnp.tile([128, nfb, T], BF, tag="ln_x2")
    nc.vector.tensor_tensor(x2, xb, xb, OP.mult)
    st = ps_rs.tile([1, 2, T], F32, tag="rs")
    for fb in range(nfb):
        nc.tensor.matmul(st[:, 0, :], ones, xb[:, fb, :],
                         start=(fb == 0), stop=(fb == nfb - 1))
    for fb in range(nfb):
        nc.tensor.matmul(st[:, 1, :], ones, x2[:, fb, :],
                         start=(fb == 0), stop=(fb == nfb - 1))
    # m = sum/D ; var = sumsq/D - m^2 ; rstd = 1/sqrt(var+eps) ; mr = m*rstd
    mv = sb.tile([1, 2, T], F32, tag="ln_mv")       # [m, ex2]
    nc.vector.tensor_scalar_mul(mv, st, 1.0 / D)
    var = sb.tile([1, T], F32, tag="ln_var")
    nc.vector.tensor_tensor(var, mv[:, 0, :], mv[:, 0, :], OP.mult)
    nc.vector.tensor_tensor(var, mv[:, 1, :], var, OP.subtract)
    eps_t = sb.tile([1, 1], F32, tag="ln_eps")
    nc.vector.memset(eps_t, float(eps))
    sd = sb.tile([1, T], F32, tag="ln_sd")
    nc.scalar.activation(sd, var, AF.Sqrt, bias=eps_t)
    rstd = sb.tile([1, T], F32, tag="ln_rstd")
    nc.vector.reciprocal(rstd, sd)
    rmr = sb.tile([1, 2, T], BF, tag="ln_rmr")      # [rstd, m*rstd] bf16
    nc.vector.tensor_copy(rmr[:, 0, :], rstd)
    nc.vector.tensor_tensor(rmr[:, 1, :], mv[:, 0, :], rstd, OP.mult)
    # broadcast to 128 partitions via rank-1 matmul
    _, ones1, _, _ = csts
    bc = ps_big.tile([128, 512], F32, tag="mm")
    nc.tensor.matmul(bc[:, 0:2 * T], ones1, rmr[:, :, :], start=True, stop=True)
    bcs = sb.tile([128, 2, T], BF, tag="ln_bcs")
    nc.vector.tensor_copy(bcs, bc[:, 0:2 * T])
    # u = x*rstd - m*rstd ; out = u*g + b (per-partition scale/bias)
    u = lnp.tile([128, nfb, T], BF, tag="ln_u")
    nc.vector.tensor_tensor(
        u, xb, bcs[:, 0, None, :].to_broadcast([128, nfb, T]), OP.mult)
    nc.vector.tensor_tensor(
        u, u, bcs[:, 1, None, :].to_broadcast([128, nfb, T]), OP.subtract)
    for fb in range(nfb):
        nc.scalar.activation(out[:, fb, :], u[:, fb, :], AF.Identity,
                             bias=b_ap[:, fb:fb + 1], scale=g_ap[:, fb:fb + 1])


def build(nlayers=NLAYERS, debug=False, gelu_fn=None):
    if gelu_fn is None:
        gelu_fn = AF.Gelu
    nc = bacc.Bacc(num_devices=N_CORES)

    # ---- I/O declarations
    def inp(name, shape, dt):
        return nc.dram_tensor(name, shape, dt, kind="ExternalInput")

    xT = inp("xT", [IN_DIM, T], F32)
    expb = inp("expb", [HL, 4, 128, T], BF)
    qkvw = inp("qkvw", [nlayers, HDIM, 512], BF)
    vw = inp("vw", [nlayers, HDIM, 256], BF)
    qkvb = inp("qkvb", [nlayers, 512], F32)
    vb = inp("vb", [nlayers, 256], F32)
    pe_wT = inp("pe_wT", [IN_DIM, HDIM], BF)
    pe_b = inp("pe_b", [HDIM], F32)
    pe_ln1_g = inp("pe_ln1_g", [IN_DIM], F32)
    pe_ln1_b = inp("pe_ln1_b", [IN_DIM], F32)
    pe_ln2_g = inp("pe_ln2_g", [HDIM], F32)
    pe_ln2_b = inp("pe_ln2_b", [HDIM], F32)
    lnA_g = inp("lnA_g", [nlayers, HDIM], F32)
    lnA_b = inp("lnA_b", [nlayers, HDIM], F32)
    lnF_g = inp("lnF_g", [nlayers, HDIM], F32)
    lnF_b = inp("lnF_b", [nlayers, HDIM], F32)
    outw = inp("outw", [nlayers, 256, HDIM], BF)
    outb = inp("outb", [nlayers, HDIM], F32)
    fc1w = inp("fc1w", [nlayers, HDIM, FF], BF)
    fc1b = inp("fc1b", [nlayers, FF], F32)
    fc2w = inp("fc2w", [nlayers, FF, HDIM], BF)
    fc2b = inp("fc2b", [nlayers, HDIM], F32)
    fin_g = inp("fin_g", [HDIM], F32)
    fin_b = inp("fin_b", [HDIM], F32)
    headw = inp("headw", [HDIM, NCLS], BF)
    headb = inp("headb", [NCLS], F32)

    out = nc.dram_tensor("out", [T, NCLS], F32, kind="ExternalOutput")
    dbg = {}
    if debug:
        for name, shape, dt in [
                ("dbg_z0", [128, 8, T], F32), ("dbg_ha", [128, 8, T], BF),
                ("dbg_hall", [128, 8, R, T], BF),
                ("dbg_qk", [128, 4, S], BF), ("dbg_v", [128, 8, 256], BF),
                ("dbg_o", [128, 2, S], BF), ("dbg_z1", [128, 8, T], F32)]:
            dbg[name] = nc.dram_tensor(name, shape, dt, kind="ExternalOutput")

    with tile.TileContext(nc) as tc:
        pools = [
            tc.tile_pool(name="persist", bufs=1),
            tc.tile_pool(name="sb", bufs=2),
            tc.tile_pool(name="big", bufs=1),
            tc.tile_pool(name="wpool", bufs=3),
            tc.tile_pool(name="wpool2", bufs=2),
            tc.tile_pool(name="attp", bufs=3),
            tc.tile_pool(name="lnp", bufs=1),
            tc.tile_pool(name="dram", bufs=1, space="DRAM"),
            tc.tile_pool(name="ps_big", bufs=3, space="PSUM"),
            tc.tile_pool(name="ps_o", bufs=2, space="PSUM"),
            tc.tile_pool(name="ps_rs", bufs=2, space="PSUM"),
        ]
        ctxs = [p.__enter__() for p in pools]
        (persist, sb, big, wpool, wpool2, attp, lnp, dram,
         ps_big, ps_o, ps_rs) = ctxs

        # ---- constants
        ones = persist.tile([128, 1], BF)
        nc.vector.memset(ones, 1.0)
        ones1 = persist.tile([1, 128], BF)
        nc.vector.memset(ones1, 1.0)
        ind_lo = persist.tile([1, 128], BF)
        nc.vector.memset(ind_lo, 0.0)
        nc.vector.memset(ind_lo[:, 0:64], 1.0)
        ind_hi = persist.tile([1, 128], BF)
        nc.vector.memset(ind_hi, 0.0)
        nc.vector.memset(ind_hi[:, 64:128], 1.0)
        ident = persist.tile([128, 128], F32)
        make_identity(nc, ident)
        csts = (ones, ones1, ind_lo, ind_hi)

        expb_sb = persist.tile([128, HL, 4, T], BF)
        nc.sync.dma_start(out=expb_sb, in_=expb.rearrange("h i p q -> p h i q"))

        def load_gb(g_dram, b_dram, nfb, tag):
            g_sb = sb.tile([128, nfb], F32, tag=f"{tag}_g")
            b_sb = sb.tile([128, nfb], F32, tag=f"{tag}_b")
            nc.sync.dma_start(out=g_sb, in_=g_dram.rearrange("(fb p) -> p fb", p=128))
            nc.sync.dma_start(out=b_sb, in_=b_dram.rearrange("(fb p) -> p fb", p=128))
            return g_sb, b_sb

        z = persist.tile([128, 8, T], F32)

        # ---- patch embed
        x_sb = big.tile([128, 10, T], F32, tag="x_sb")
        nc.sync.dma_start(out=x_sb, in_=xT.rearrange("(fb p) t -> p fb t", p=128))
        g1, b1 = load_gb(pe_ln1_g, pe_ln1_b, 10, "peln1")
        h0 = big.tile([128, 10, T], BF, tag="h0")
        _layernorm(nc, sb, lnp, ps_big, ps_rs, x_sb, h0, g1, b1, 1e-6, 10, csts)
        peb_sb, _unused = load_gb(pe_b, pe_b, 8, "peb")
        zpre = big.tile([128, 8, T], F32, tag="zpre")
        for ob in range(8):
            pw = wpool.tile([128, 10, 128], BF, tag="wo")
            nc.sync.dma_start(
                out=pw, in_=pe_wT[:, ob * 128:(ob + 1) * 128]
                    .rearrange("(ib p) o -> p ib o", p=128))
            pp = ps_big.tile([128, 512], F32, tag="mm")
            for ib in range(10):
                nc.tensor.matmul(pp[:, 0:T], pw[:, ib, :],
                                 h0[:, ib, :], start=(ib == 0), stop=(ib == 9))
            nc.scalar.activation(zpre[:, ob, :], pp[:, 0:T], AF.Identity,
                                 bias=peb_sb[:, ob:ob + 1])
        g2, b2 = load_gb(pe_ln2_g, pe_ln2_b, 8, "peln2")
        _layernorm(nc, sb, lnp, ps_big, ps_rs, zpre, z, g2, b2, 1e-6, 8, csts)
        if debug:
            nc.sync.dma_start(out=dbg["dbg_z0"][:, :, :], in_=z)

        # ---- layers
        for l in range(nlayers):
            # lnA -> ha (bf16)
            ga, ba = load_gb(lnA_g[l], lnA_b[l], 8, "lnA")
            ha = big.tile([128, 8, T], BF, tag="hx")
            _layernorm(nc, sb, lnp, ps_big, ps_rs, z, ha, ga, ba, 1e-5, 8, csts)
            if debug and l == 0:
                nc.sync.dma_start(out=dbg["dbg_ha"][:, :, :], in_=ha)
            # AllGather h
            agh_in = dram.tile([HDIM, T], BF, tag="agh_in")
            agh_out = dram.tile([R, HDIM, T], BF, tag="agh_out")
            nc.sync.dma_start(out=agh_in.rearrange("(fb p) t -> p fb t", p=128),
                              in_=ha)
            nc.gpsimd.collective_compute(
                "AllGather", OP.bypass, replica_groups=GROUPS,
                ins=[agh_in.opt()], outs=[agh_out.opt()])
            h_all = big.tile([128, 8, R, T], BF, tag="h_all")
            for r in range(R):
                nc.sync.dma_start(
                    out=h_all[:, :, r, :],
                    in_=agh_out[r].rearrange("(fb p) t -> p fb t", p=128))
            if debug and l == 0:
                nc.sync.dma_start(out=dbg["dbg_hall"][:, :, :, :], in_=h_all)

            # qk projections: [128 if, 8 ifb, (r t)] x w -> qk_sb [128, 4 ofb, S]
            wqk = wpool2.tile([128, 8, 512], BF, tag="wqk")
            nc.sync.dma_start(out=wqk,
                              in_=qkvw[l].rearrange("(ib p) o -> p ib o", p=128))
            qkvb_sb, _uqb = load_gb(qkvb[l], qkvb[l], 4, "qkvb")
            vb_row = sb.tile([1, 256], F32, tag="vbrow")
            nc.sync.dma_start(out=vb_row, in_=vb[l][None, :])
            vb_rowb = sb.tile([1, 256], BF, tag="vbrowb")
            nc.vector.tensor_copy(vb_rowb, vb_row)
            qk_sb = big.tile([128, 4, S], BF, tag="qk_sb")
            hflat = h_all  # [128, 8, R, T] ; token chunks of 512 = 2 r-blocks
            for ob in range(4):
                for tch in range(2):
                    pp = ps_big.tile([128, 512], F32, tag="mm")
                    for ib in range(8):
                        nc.tensor.matmul(
                            pp, wqk[:, ib, ob * 128:(ob + 1) * 128],
                            hflat[:, ib, 2 * tch:2 * tch + 2, :],
                            start=(ib == 0), stop=(ib == 7))
                    nc.scalar.activation(qk_sb[:, ob, 512 * tch:512 * (tch + 1)],
                                         pp, AF.Identity, bias=qkvb_sb[:, ob:ob + 1])
            if debug and l == 0:
                nc.sync.dma_start(out=dbg["dbg_qk"][:, :, :], in_=qk_sb)

            # v projection: token-major v_sb [128 tok, 8 tb, 256 vf]
            wv = big.tile([128, 8, 256], BF, tag="wv")
            nc.sync.dma_start(out=wv,
                              in_=vw[l].rearrange("(ib p) o -> p ib o", p=128))
            v_sb = big.tile([128, 8, 256], BF, tag="v_sb")
            for tb in range(8):
                r, half = tb // 2, tb % 2
                pp = ps_big.tile([128, 512], F32, tag="mm")
                for ib in range(8):
                    nc.tensor.matmul(
                        pp[:, 0:256],
                        hflat[:, ib, r, half * 128:(half + 1) * 128],
                        wv[:, ib, :], start=(ib == 0), stop=False)
                nc.tensor.matmul(pp[:, 0:256], ones1, vb_rowb,
                                 start=False, stop=True)
                nc.vector.tensor_copy(v_sb[:, tb, :], pp[:, 0:256])
            if debug and l == 0:
                nc.sync.dma_start(out=dbg["dbg_v"][:, :, :], in_=v_sb)

            # attention, head pairs
            o_all = big.tile([128, 2, S], BF, tag="o_all")
            for hp in range(2):
                h0i, h1i = 2 * hp, 2 * hp + 1   # local head indices
                qofb, kofb = hp, 2 + hp
                for qc in range(4):
                    nk = 2 * qc + 2
                    o_ps = ps_o.tile([128, T], F32, tag="o")
                    rs = ps_rs.tile([1, 2, T], F32, tag="rs")
                    for kblk in range(nk):
                        idx = max(kblk - 2 * qc + 2, 0)
                        s0 = ps_big.tile([128, 512], F32, tag="mm")
                        nc.tensor.matmul(
                            s0[:, 0:T],
                            qk_sb[0:64, kofb, kblk * 128:(kblk + 1) * 128],
                            qk_sb[0:64, qofb, qc * T:(qc + 1) * T],
                            start=True, stop=True)
                        s1 = ps_big.tile([128, 512], F32, tag="mm")
                        nc.tensor.matmul(
                            s1[:, 0:T],
                            qk_sb[64:128, kofb, kblk * 128:(kblk + 1) * 128],
                            qk_sb[64:128, qofb, qc * T:(qc + 1) * T],
                            start=True, stop=True, tile_position=(64, 0))
                        pT = attp.tile([128, 2, T], BF, tag="pT")
                        e0 = attp.tile([128, 2, T], BF, tag="e0")
                        nc.scalar.activation(e0[:, 0, :], s0[:, 0:T], AF.Exp,
                                             scale=float(SCALE))
                        nc.scalar.activation(e0[:, 1, :], s1[:, 0:T], AF.Exp,
                                             scale=float(SCALE))
                        nc.vector.tensor_tensor(
                            pT[:, 0, :], e0[:, 0, :], expb_sb[:, h0i, idx, :],
                            OP.mult)
                        nc.vector.tensor_tensor(
                            pT[:, 1, :], e0[:, 1, :], expb_sb[:, h1i, idx, :],
                            OP.mult)
                        nc.tensor.matmul(rs, ones, pT,
                                         start=(kblk == 0), stop=(kblk == nk - 1))
                        nc.tensor.matmul(
                            o_ps[0:64, :], v_sb[:, kblk, h0i * 64:(h0i + 1) * 64],
                            pT[:, 0, :], start=(kblk == 0), stop=(kblk == nk - 1),
                            skip_group_check=True)
                        nc.tensor.matmul(
                            o_ps[64:128, :], v_sb[:, kblk, h1i * 64:(h1i + 1) * 64],
                            pT[:, 1, :], start=(kblk == 0), stop=(kblk == nk - 1),
                            tile_position=(0, 64), skip_group_check=True)
                    rc = sb.tile([1, 2, T], F32, tag="rc")
                    nc.vector.reciprocal(rc, rs)
                    rcb = sb.tile([1, 2, T], BF, tag="rcb")
                    nc.vector.tensor_copy(rcb, rc)
                    bc = ps_big.tile([128, 512], F32, tag="mm")
                    nc.tensor.matmul(bc[:, 0:T], ind_lo, rcb[:, 0, :],
                                     start=True, stop=False)
                    nc.tensor.matmul(bc[:, 0:T], ind_hi, rcb[:, 1, :],
                                     start=False, stop=True)
                    bcs = sb.tile([128, T], BF, tag="bcs")
                    nc.scalar.copy(bcs, bc[:, 0:T])
                    nc.vector.tensor_tensor(
                        o_all[:, hp, qc * T:(qc + 1) * T], o_ps, bcs, OP.mult)
            if debug and l == 0:
                nc.sync.dma_start(out=dbg["dbg_o"][:, :, :], in_=o_all)

            # out-proj partials (my head features x all tokens) -> ReduceScatter
            wo_sh = big.tile([128, 2, HDIM], BF, tag="wo_sh")
            nc.sync.dma_start(
                out=wo_sh, in_=outw[l].rearrange("(ib p) o -> p ib o", p=128))
            rs_in = dram.tile([R, HDIM, T], BF, tag="rs_in")
            rs_out = dram.tile([HDIM, T], BF, tag="rs_out")
            rs_in_v = rs_in.rearrange("r (ob p) t -> p ob r t", p=128)
            for ob in range(8):
                for tch in range(2):
                    pp = ps_big.tile([128, 512], F32, tag="mm")
                    for ib in range(2):
                        nc.tensor.matmul(
                            pp, wo_sh[:, ib, ob * 128:(ob + 1) * 128],
                            o_all[:, ib, tch * 512:(tch + 1) * 512],
                            start=(ib == 0), stop=(ib == 1))
                    zp = sb.tile([128, 2, T], BF, tag="zp")
                    nc.scalar.copy(zp, pp)
                    nc.sync.dma_start(
                        out=rs_in_v[:, ob, 2 * tch:2 * tch + 2, :], in_=zp)
            nc.gpsimd.collective_compute(
                "ReduceScatter", OP.add, replica_groups=GROUPS,
                ins=[rs_in.opt()], outs=[rs_out.opt()])
            zr = big.tile([128, 8, T], BF, tag="zr")
            nc.sync.dma_start(out=zr,
                              in_=rs_out.rearrange("(ob p) t -> p ob t", p=128))
            outb_sb, _u2 = load_gb(outb[l], outb[l], 8, "outb")
            for ob in range(8):
                nc.vector.scalar_tensor_tensor(
                    z[:, ob, :], zr[:, ob, :], outb_sb[:, ob:ob + 1], z[:, ob, :],
                    OP.add, OP.add)

            # lnF -> hf
            gf, bf_ = load_gb(lnF_g[l], lnF_b[l], 8, "lnF")
            hf = big.tile([128, 8, T], BF, tag="hx")
            _layernorm(nc, sb, lnp, ps_big, ps_rs, z, hf, gf, bf_, 1e-5, 8, csts)
            # fc1 + gelu
            f1b_sb = sb.tile([128, 32], F32, tag="f1b")
            nc.sync.dma_start(out=f1b_sb,
                              in_=fc1b[l].rearrange("(fb p) -> p fb", p=128))
            gact = big.tile([128, 32, T], BF, tag="gact")
            for ob in range(32):
                wf = wpool.tile([128, 8, 128], BF, tag="wo")
                nc.sync.dma_start(
                    out=wf,
                    in_=fc1w[l][:, ob * 128:(ob + 1) * 128]
                        .rearrange("(ib p) o -> p ib o", p=128))
                pp = ps_big.tile([128, 512], F32, tag="mm")
                for ib in range(8):
                    nc.tensor.matmul(pp[:, 0:T], wf[:, ib, :], hf[:, ib, :],
                                     start=(ib == 0), stop=(ib == 7))
                nc.scalar.activation(gact[:, ob, :], pp[:, 0:T], gelu_fn,
                                     bias=f1b_sb[:, ob:ob + 1])
            # fc2 += z
            f2b_sb, _u3 = load_gb(fc2b[l], fc2b[l], 8, "f2b")
            for ob in range(8):
                wf2 = wpool2.tile([128, 32, 128], BF, tag="wf2")
                nc.sync.dma_start(
                    out=wf2,
                    in_=fc2w[l][:, ob * 128:(ob + 1) * 128]
                        .rearrange("(ib p) o -> p ib o", p=128))
                pp = ps_big.tile([128, 512], F32, tag="mm")
                for ib in range(32):
                    nc.tensor.matmul(pp[:, 0:T], wf2[:, ib, :], gact[:, ib, :],
                                     start=(ib == 0), stop=(ib == 31))
                nc.vector.scalar_tensor_tensor(
                    z[:, ob, :], pp[:, 0:T], f2b_sb[:, ob:ob + 1], z[:, ob, :],
                    OP.add, OP.add)
            if debug and l == 0:
                nc.sync.dma_start(out=dbg["dbg_z1"][:, :, :], in_=z)

        # ---- final LN + head
        gg, bb = load_gb(fin_g, fin_b, 8, "fin")
        hfin = big.tile([128, 8, T], BF, tag="hx")
        _layernorm(nc, sb, lnp, ps_big, ps_rs, z, hfin, gg, bb, 1e-5, 8, csts)
        hw_sb = big.tile([128, 8, NCLS], BF, tag="hw")
        nc.sync.dma_start(out=hw_sb,
                          in_=headw.rearrange("(ib p) o -> p ib o", p=128))
        hb_sb = sb.tile([NCLS, 1], F32, tag="hb")
        nc.sync.dma_start(out=hb_sb, in_=headb[:, None])
        lp = ps_big.tile([128, 512], F32, tag="mm")
        for ib in range(8):
            nc.tensor.matmul(lp[0:NCLS, 0:T], hw_sb[:, ib, :], hfin[:, ib, :],
                             start=(ib == 0), stop=(ib == 7))
        logit_fm = big.tile([NCLS, T], F32, tag="logit")
        nc.scalar.activation(logit_fm, lp[0:NCLS, 0:T], AF.Identity,
                             bias=hb_sb[:, 0:1])
        # transpose to token-major and store
        osb = big.tile([128, 2, NCLS], F32, tag="osb")
        for ch in range(2):
            tp = ps_big.tile([128, 512], F32, tag="mm")
            nc.tensor.transpose(tp[:, 0:NCLS],
                                logit_fm[:, ch * 128:(ch + 1) * 128],
                                ident[0:NCLS, 0:NCLS])
            nc.vector.tensor_copy(osb[:, ch, :], tp[:, 0:NCLS])
        nc.sync.dma_start(out=out.rearrange("(c p) n -> p c n", p=128), in_=osb)

        for p in reversed(pools):
            p.__exit__(None, None, None)
    nc.compile()
    return nc


def prep_inputs(inputs, nlayers=NLAYERS):
    """Host-side: shard + transpose + cast. Returns in_maps (list of 8 dicts)."""
    bf = ml_dtypes.bfloat16
    f32 = np.float32

    x = np.asarray(inputs["x"], f32)            # (2, 5120, 256)
    qkv_w = np.asarray(inputs["qkv_w"], f32)    # (L, 3072, 1024)
    rel = np.asarray(inputs["rel_table"], f32)  # (16, 257)

    shared = {
        "pe_wT": np.ascontiguousarray(np.asarray(inputs["pe_w"], f32).T).astype(bf),
        "pe_b": np.asarray(inputs["pe_b"], f32),
        "pe_ln1_g": np.asarray(inputs["pe_ln1_g"], f32),
        "pe_ln1_b": np.asarray(inputs["pe_ln1_b"], f32),
        "pe_ln2_g": np.asarray(inputs["pe_ln2_g"], f32),
        "pe_ln2_b": np.asarray(inputs["pe_ln2_b"], f32),
        "lnA_g": np.asarray(inputs["lnA_g"], f32)[:nlayers],
        "lnA_b": np.asarray(inputs["lnA_b"], f32)[:nlayers],
        "lnF_g": np.asarray(inputs["lnF_g"], f32)[:nlayers],
        "lnF_b": np.asarray(inputs["lnF_b"], f32)[:nlayers],
        "outb": np.asarray(inputs["out_b"], f32)[:nlayers],
        "fc1w": np.ascontiguousarray(
            np.asarray(inputs["fc1_w"], f32)[:nlayers].transpose(0, 2, 1)).astype(bf),
        "fc1b": np.asarray(inputs["fc1_b"], f32)[:nlayers],
        "fc2w": np.ascontiguousarray(
            np.asarray(inputs["fc2_w"], f32)[:nlayers].transpose(0, 2, 1)).astype(bf),
        "fc2b": np.asarray(inputs["fc2_b"], f32)[:nlayers],
        "fin_g": np.asarray(inputs["final_g"], f32),
        "fin_b": np.asarray(inputs["final_b"], f32),
        "headw": np.ascontiguousarray(np.asarray(inputs["head_w"], f32).T).astype(bf),
        "headb": np.asarray(inputs["head_b"], f32),
    }

    # per-position (p in 0..3) sharded tensors
    per_pos = []
    for p in range(R):
        heads = range(HL * p, HL * p + HL)
        qrows = np.concatenate([np.arange(h * 64, h * 64 + 64) for h in heads])
        krows = HDIM + qrows
        vrows = 2 * HDIM + qrows
        sel_qk = np.concatenate([qrows, krows])
        qkvw_p = np.ascontiguousarray(
            qkv_w[:nlayers, sel_qk, :].transpose(0, 2, 1)).astype(bf)
        vw_p = np.ascontiguousarray(
            qkv_w[:nlayers, vrows, :].transpose(0, 2, 1)).astype(bf)
        qkv_b = np.asarray(inputs["qkv_b"], f32)
        qkvb_p = qkv_b[:nlayers][:, sel_qk].copy()
        vb_p = qkv_b[:nlayers][:, vrows].copy()
        # expb tiles: idx 0 = const (d<=-256); idx 1..3: d = -128, 0, 128
        expb_p = np.zeros((HL, 4, 128, T), np.float32)
        kk = np.arange(128)[:, None]
        qq = np.arange(T)[None, :]
        for hh, h in enumerate(heads):
            expb_p[hh, 0] = np.exp(rel[h, 0])
            for i, d in enumerate([-128, 0, 128]):
                diff = d + kk - qq
                val = np.exp(rel[h, np.clip(diff, -MAX_REL, MAX_REL) + MAX_REL])
                val[diff > 0] = 0.0
                expb_p[hh, 1 + i] = val
        out_w = np.asarray(inputs["out_w"], f32)
        outw_p = np.ascontiguousarray(
            out_w[:nlayers].transpose(0, 2, 1)[:, qrows, :]).astype(bf)
        per_pos.append({
            "qkvw": qkvw_p, "vw": vw_p, "qkvb": qkvb_p, "vb": vb_p,
            "expb": expb_p.astype(bf), "outw": outw_p,
        })

    in_maps = []
    for c in range(N_CORES):
        b, p = c // R, c % R
        xs = x[b, p * T * TIN:(p + 1) * T * TIN, :]        # (1280, 256)
        xTc = np.ascontiguousarray(
            xs.reshape(T, IN_DIM).T)                        # (1280, 256)
        m = {"xT": xTc.astype(f32)}
        m.update(per_pos[p])
        m.update(shared)
        in_maps.append(m)
    return in_maps


_BUILD_CACHE = {}


def _assemble(results):
    B = 2
    out = np.zeros((B, S, NCLS), np.float32)
    for c in range(N_CORES):
        b, p = c // R, c % R
        out[b, p * T:(p + 1) * T, :] = results[c]["out"]
    return out


def kernel(**inputs) -> np.ndarray:
    in_maps = prep_inputs(inputs)
    key = NLAYERS
    if key not in _BUILD_CACHE:
        _BUILD_CACHE[key] = build(NLAYERS)
    nc = _BUILD_CACHE[key]
    res = run_bass_kernel_spmd(nc, in_maps, core_ids=list(range(N_CORES)))
    return _assemble(res.results)


class Executor:
    """jax.jit shard_map executor with device-resident inputs (for timing)."""

    def __init__(self, nc, in_maps):
        import jax
        from jax.sharding import Mesh, NamedSharding, PartitionSpec
        from jax.experimental.shard_map import shard_map
        import concourse.mybir as mybir
        from concourse import bass2jax
        self.jax = jax
        bass2jax.install_neuronx_cc_hook()
        pname = nc.partition_id_tensor.name if nc.partition_id_tensor else None
        in_names, out_names, out_avals, zero_outs = [], [], [], []
        for alloc in nc.m.functions[0].allocations:
            if not isinstance(alloc, mybir.MemoryLocationSet):
                continue
            name = alloc.memorylocations[0].name
            if alloc.kind == "ExternalInput":
                if name != pname:
                    in_names.append(name)
            elif alloc.kind == "ExternalOutput":
                out_names.append(name)
                shape = tuple(alloc.tensor_shape)
                dtype = mybir.dt.np(alloc.dtype)
                out_avals.append(jax.core.ShapedArray(shape, dtype))
                zero_outs.append(np.zeros(shape, dtype))
        n_params = len(in_names)
        n_outs = len(out_avals)
        names_all = list(in_names) + out_names + ([pname] if pname else [])

        def _body(*args):
            operands = list(args)
            if pname is not None:
                operands.append(bass2jax.partition_id_tensor())
            return tuple(bass2jax._bass_exec_p.bind(
                *operands, out_avals=tuple(out_avals), in_names=tuple(names_all),
                out_names=tuple(out_names), lowering_input_output_aliases=(),
                sim_require_finite=True, sim_require_nnan=True, nc=nc))

        devices = jax.devices()[:N_CORES]
        mesh = Mesh(np.asarray(devices), ("core",))
        spec = PartitionSpec("core")
        self.sharded = jax.jit(
            shard_map(_body, mesh=mesh, in_specs=(spec,) * (n_params + n_outs),
                      out_specs=(spec,) * n_outs, check_rep=False),
            donate_argnums=tuple(range(n_params, n_params + n_outs)),
            keep_unused=True)
        per_core = [[np.asarray(m[nm]) for nm in in_names] for m in in_maps]
        sh = NamedSharding(mesh, spec)
        self.dev_in = [jax.device_put(
            np.concatenate([per_core[c][i] for c in range(N_CORES)], 0), sh)
            for i in range(n_params)]
        self.zero_outs = zero_outs
        self.sh = sh
        self.out_names = out_names

    def _zeros(self):
        return [self.jax.device_put(
            np.zeros((N_CORES * z.shape[0], *z.shape[1:]), z.dtype), self.sh)
            for z in self.zero_outs]

    def run(self):
        out = self.sharded(*self.dev_in, *self._zeros())
        self.jax.block_until_ready(out)
        return out

    def time(self, iters=5):
        import time as _time
        self.run()
        self.run()
        best = float("inf")
        for _ in range(iters):
            zo = self._zeros()
            self.jax.block_until_ready(zo)
            t0 = _time.perf_counter()
            out = self.sharded(*self.dev_in, *zo)
            self.jax.block_until_ready(out)
            best = min(best, _time.perf_counter() - t0)
        return best, out


# revision 21
# speedup vs baseline: 34.7922x; 1.0021x over previous
"""Trainium2 Bass kernel for nn_CausalTransformer (B=2, S=1024, H=1024, 16 heads,
8 layers, FF=4096, rel-pos bias + causal mask, patch embed TIN=5).

Distribution over 8 NeuronCores: two groups of 4 (one per batch element).
Within a group the residual stream is sequence-sharded (256 tokens/core,
feature-major [feature_partition, token] layout); attention is head-sharded
(4 heads/core). Two 4-core AllGathers per layer (post-lnA activations and
attention output, both bf16). All matmuls run in bf16 with fp32 PSUM
accumulation; the residual stream stays fp32 in SBUF.
"""
import sys

if "/opt/trn_rl_repo" not in sys.path:
    sys.path.insert(0, "/opt/trn_rl_repo")

import numpy as np
import ml_dtypes

import concourse.bacc as bacc
import concourse.mybir as mybir
import concourse.tile as tile
from concourse.masks import make_identity
from concourse.bass_utils import run_bass_kernel_spmd

BF = mybir.dt.bfloat16
F32 = mybir.dt.float32
AF = mybir.ActivationFunctionType
OP = mybir.AluOpType

N_CORES = 8
GROUPS = [[0, 1, 2, 3], [4, 5, 6, 7]]
R = 4              # ranks per group
T = 256            # tokens per core
S = 1024           # tokens per batch
HDIM = 1024
N_HEADS = 16
HL = 4             # heads per core
HEAD_DIM = 64
FF = 4096
NLAYERS = 8
TIN = 5
FEAT = 256
IN_DIM = TIN * FEAT  # 1280
NCLS = 100
MAX_REL = 128
SCALE = 1.0 / np.sqrt(HEAD_DIM)


def _layernorm(nc, sb, lnp, ps_big, ps_rs, x, out, g_ap, b_ap, eps, nfb, csts):
    """Feature-major layernorm: x [128, nfb, T] fp32 -> out (bf16 or f32).

    Stats via bf16 cast + ones-matmul partition reduction; apply with
    per-partition g/b on the scalar engine. g_ap/b_ap: SBUF [128, nfb]."""
    ones, _, _, _ = csts
    D = nfb * 128
    xb = lnp.tile([128, nfb, T], BF, tag="ln_xb")
    x2 = lnp.tile([128, nfb, T], BF, tag="ln_x2")
    st = ps_rs.tile([1, 2, T], F32, tag="rs")
    # per-fb cast/square so stats overlap with the producer of x (fc2/outproj)
    for fb in range(nfb):
        nc.vector.tensor_copy(xb[:, fb, :], x[:, fb, :])
        nc.vector.tensor_tensor(x2[:, fb, :], xb[:, fb, :], xb[:, fb, :], OP.mult)
    for fb in range(nfb):
        nc.tensor.matmul(st[:, 0, :], ones, xb[:, fb, :],
                         start=(fb == 0), stop=(fb == nfb - 1))
    for fb in range(nfb):
        nc.tensor.matmul(st[:, 1, :], ones, x2[:, fb, :],
                         start=(fb == 0), stop=(fb == nfb - 1))
    # m = sum/D ; var = sumsq/D - m^2 ; rstd = 1/sqrt(var+eps) ; mr = m*rstd
    mv = sb.tile([1, 2, T], F32, tag="ln_mv")       # [m, ex2]
    nc.vector.tensor_scalar_mul(mv, st, 1.0 / D)
    var = sb.tile([1, T], F32, tag="ln_var")
    nc.vector.tensor_tensor(var, mv[:, 0, :], mv[:, 0, :], OP.mult)
    nc.vector.tensor_tensor(var, mv[:, 1, :], var, OP.subtract)
    eps_t = sb.tile([1, 1], F32, tag="ln_eps")
    nc.vector.memset(eps_t, float(eps))
    sd = sb.tile([1, T], F32, tag="ln_sd")
    nc.scalar.activation(sd, var, AF.Sqrt, bias=eps_t)
    rstd = sb.tile([1, T], F32, tag="ln_rstd")
    nc.vector.reciprocal(rstd, sd)
    rmr = sb.tile([1, 2, T], BF, tag="ln_rmr")      # [rstd, m*rstd] bf16
    nc.vector.tensor_copy(rmr[:, 0, :], rstd)
    nc.vector.tensor_tensor(rmr[:, 1, :], mv[:, 0, :], rstd, OP.mult)
    # broadcast to 128 partitions via rank-1 matmul
    _, ones1, _, _ = csts
    bc = ps_big.tile([128, 2, T], F32, tag="mm")
    nc.tensor.matmul(bc, ones1, rmr[:, :, :], start=True, stop=True)
    # u = x*rstd - m*rstd ; out = u*g + b (per-partition scale/bias)
    u = lnp.tile([128, nfb, T], BF, tag="ln_u")
    nc.vector.tensor_tensor(
        u, xb, bc[:, 0, None, :].to_broadcast([128, nfb, T]), OP.mult)
    nc.vector.tensor_tensor(
        u, u, bc[:, 1, None, :].to_broadcast([128, nfb, T]), OP.subtract)
    for fb in range(nfb):
        nc.scalar.activation(out[:, fb, :], u[:, fb, :], AF.Identity,
                             bias=b_ap[:, fb:fb + 1], scale=g_ap[:, fb:fb + 1])


def build(nlayers=NLAYERS, debug=False, gelu_fn=None, no_coll=False, skip=()):
    if gelu_fn is None:
        gelu_fn = AF.Gelu
    nc = bacc.Bacc(num_devices=N_CORES)

    # ---- I/O declarations
    def inp(name, shape, dt):
        return nc.dram_tensor(name, shape, dt, kind="ExternalInput")

    xT = inp("xT", [IN_DIM, T], F32)
    expb = inp("expb", [HL, 4, 128, T], BF)
    qkvw = inp("qkvw", [nlayers, HDIM, 512], BF)
    vw = inp("vw", [nlayers, HDIM, 256], BF)
    qkvb = inp("qkvb", [nlayers, 512], F32)
    vb = inp("vb", [nlayers, 256], F32)
    pe_wT = inp("pe_wT", [IN_DIM, HDIM], BF)
    pe_b = inp("pe_b", [HDIM], F32)
    pe_ln1_g = inp("pe_ln1_g", [IN_DIM], F32)
    pe_ln1_b = inp("pe_ln1_b", [IN_DIM], F32)
    pe_ln2_g = inp("pe_ln2_g", [HDIM], F32)
    pe_ln2_b = inp("pe_ln2_b", [HDIM], F32)
    lnA_g = inp("lnA_g", [nlayers, HDIM], F32)
    lnA_b = inp("lnA_b", [nlayers, HDIM], F32)
    lnF_g = inp("lnF_g", [nlayers, HDIM], F32)
    lnF_b = inp("lnF_b", [nlayers, HDIM], F32)
    outw = inp("outw", [nlayers, 256, HDIM], BF)
    outb = inp("outb", [nlayers, HDIM], F32)
    fc1w = inp("fc1w", [nlayers, HDIM, FF], BF)
    fc1b = inp("fc1b", [nlayers, FF], F32)
    fc2w = inp("fc2w", [nlayers, FF, HDIM], BF)
    fc2b = inp("fc2b", [nlayers, HDIM], F32)
    fin_g = inp("fin_g", [HDIM], F32)
    fin_b = inp("fin_b", [HDIM], F32)
    headw = inp("headw", [HDIM, NCLS], BF)
    headb = inp("headb", [NCLS], F32)

    out = nc.dram_tensor("out", [T, NCLS], F32, kind="ExternalOutput")
    dbg = {}
    if debug:
        for name, shape, dt in [
                ("dbg_z0", [128, 8, T], F32), ("dbg_ha", [128, 8, T], BF),
                ("dbg_hall", [128, 8, R, T], BF),
                ("dbg_qk", [128, 4, S], BF), ("dbg_v", [128, 8, 256], BF),
                ("dbg_o", [128, 2, S], BF), ("dbg_z1", [128, 8, T], F32)]:
            dbg[name] = nc.dram_tensor(name, shape, dt, kind="ExternalOutput")

    with tile.TileContext(nc) as tc:
        pools = [
            tc.tile_pool(name="persist", bufs=1),
            tc.tile_pool(name="sb", bufs=2),
            tc.tile_pool(name="big", bufs=1),
            tc.tile_pool(name="wpool", bufs=3),
            tc.tile_pool(name="wpool2", bufs=2),
            tc.tile_pool(name="attp", bufs=3),
            tc.tile_pool(name="lnp", bufs=1),
            tc.tile_pool(name="dram", bufs=1, space="DRAM"),
            tc.tile_pool(name="ps_big", bufs=3, space="PSUM"),
            tc.tile_pool(name="ps_o", bufs=2, space="PSUM"),
            tc.tile_pool(name="ps_rs", bufs=2, space="PSUM"),
        ]
        ctxs = [p.__enter__() for p in pools]
        (persist, sb, big, wpool, wpool2, attp, lnp, dram,
         ps_big, ps_o, ps_rs) = ctxs

        # ---- constants
        ones = persist.tile([128, 1], BF)
        nc.vector.memset(ones, 1.0)
        ones1 = persist.tile([1, 128], BF)
        nc.vector.memset(ones1, 1.0)
        ind_lo = persist.tile([1, 128], BF)
        nc.vector.memset(ind_lo, 0.0)
        nc.vector.memset(ind_lo[:, 0:64], 1.0)
        ind_hi = persist.tile([1, 128], BF)
        nc.vector.memset(ind_hi, 0.0)
        nc.vector.memset(ind_hi[:, 64:128], 1.0)
        ident = persist.tile([128, 128], F32)
        make_identity(nc, ident)
        csts = (ones, ones1, ind_lo, ind_hi)

        expb_sb = persist.tile([128, HL, 4, T], BF)
        nc.sync.dma_start(out=expb_sb, in_=expb.rearrange("h i p q -> p h i q"))

        def load_gb(g_dram, b_dram, nfb, tag):
            g_sb = sb.tile([128, nfb], F32, tag=f"{tag}_g")
            b_sb = sb.tile([128, nfb], F32, tag=f"{tag}_b")
            nc.sync.dma_start(out=g_sb, in_=g_dram.rearrange("(fb p) -> p fb", p=128))
            nc.sync.dma_start(out=b_sb, in_=b_dram.rearrange("(fb p) -> p fb", p=128))
            return g_sb, b_sb

        z = persist.tile([128, 8, T], F32)

        # ---- patch embed
        x_sb = big.tile([128, 10, T], F32, tag="x_sb")
        nc.sync.dma_start(out=x_sb, in_=xT.rearrange("(fb p) t -> p fb t", p=128))
        g1, b1 = load_gb(pe_ln1_g, pe_ln1_b, 10, "peln1")
        h0 = big.tile([128, 10, T], BF, tag="h0")
        _layernorm(nc, sb, lnp, ps_big, ps_rs, x_sb, h0, g1, b1, 1e-6, 10, csts)
        peb_sb, _unused = load_gb(pe_b, pe_b, 8, "peb")
        zpre = big.tile([128, 8, T], F32, tag="zpre")
        for ob in range(8):
            pw = wpool.tile([128, 10, 128], BF, tag="wo")
            nc.sync.dma_start(
                out=pw, in_=pe_wT[:, ob * 128:(ob + 1) * 128]
                    .rearrange("(ib p) o -> p ib o", p=128))
            pp = ps_big.tile([128, 512], F32, tag="mm")
            for ib in range(10):
                nc.tensor.matmul(pp[:, 0:T], pw[:, ib, :],
                                 h0[:, ib, :], start=(ib == 0), stop=(ib == 9))
            nc.scalar.activation(zpre[:, ob, :], pp[:, 0:T], AF.Identity,
                                 bias=peb_sb[:, ob:ob + 1])
        g2, b2 = load_gb(pe_ln2_g, pe_ln2_b, 8, "peln2")
        _layernorm(nc, sb, lnp, ps_big, ps_rs, zpre, z, g2, b2, 1e-6, 8, csts)
        if debug:
            nc.sync.dma_start(out=dbg["dbg_z0"][:, :, :], in_=z)

        # ---- layers
        for l in range(nlayers):
            # lnA -> ha (bf16)
            ga, ba = load_gb(lnA_g[l], lnA_b[l], 8, "lnA")
            ha = big.tile([128, 8, T], BF, tag="hx")
            _layernorm(nc, sb, lnp, ps_big, ps_rs, z, ha, ga, ba, 1e-5, 8, csts)
            if debug and l == 0:
                nc.sync.dma_start(out=dbg["dbg_ha"][:, :, :], in_=ha)
            # AllGather h
            agh_in = dram.tile([HDIM, T], BF, tag="agh_in")
            agh_out = dram.tile([R, HDIM, T], BF, tag="agh_out")
            nc.sync.dma_start(out=agh_in.rearrange("(fb p) t -> p fb t", p=128),
                              in_=ha)
            if no_coll:
                for r in range(R):
                    nc.sync.dma_start(out=agh_out[r], in_=agh_in[:, :])
            else:
                nc.gpsimd.collective_compute(
                    "AllGather", OP.bypass, replica_groups=GROUPS,
                    ins=[agh_in.opt()], outs=[agh_out.opt()])
            h_all = big.tile([128, 8, R, T], BF, tag="h_all")
            for r in range(R):
                nc.sync.dma_start(
                    out=h_all[:, :, r, :],
                    in_=agh_out[r].rearrange("(fb p) t -> p fb t", p=128))
            if debug and l == 0:
                nc.sync.dma_start(out=dbg["dbg_hall"][:, :, :, :], in_=h_all)

            # qk projections: [128 if, 8 ifb, (r t)] x w -> qk_sb [128, 4 ofb, S]
            wqk = wpool2.tile([128, 8, 512], BF, tag="wqk")
            nc.sync.dma_start(out=wqk,
                              in_=qkvw[l].rearrange("(ib p) o -> p ib o", p=128))
            qkvb_sb, _uqb = load_gb(qkvb[l], qkvb[l], 4, "qkvb")
            vb_row = sb.tile([1, 256], F32, tag="vbrow")
            nc.sync.dma_start(out=vb_row, in_=vb[l][None, :])
            vb_rowb = sb.tile([1, 256], BF, tag="vbrowb")
            nc.vector.tensor_copy(vb_rowb, vb_row)
            qk_sb = big.tile([128, 4, S], BF, tag="qk_sb")
            hflat = h_all  # [128, 8, R, T] ; token chunks of 512 = 2 r-blocks
            for ob in range(4):
                for tch in range(2):
                    pp = ps_big.tile([128, 512], F32, tag="mm")
                    for ib in range(8):
                        nc.tensor.matmul(
                            pp, wqk[:, ib, ob * 128:(ob + 1) * 128],
                            hflat[:, ib, 2 * tch:2 * tch + 2, :],
                            start=(ib == 0), stop=(ib == 7))
                    nc.scalar.activation(qk_sb[:, ob, 512 * tch:512 * (tch + 1)],
                                         pp, AF.Identity, bias=qkvb_sb[:, ob:ob + 1])
            if debug and l == 0:
                nc.sync.dma_start(out=dbg["dbg_qk"][:, :, :], in_=qk_sb)

            # v projection: token-major v_sb [128 tok, 8 tb, 256 vf]
            wv = big.tile([128, 8, 256], BF, tag="wv")
            nc.sync.dma_start(out=wv,
                              in_=vw[l].rearrange("(ib p) o -> p ib o", p=128))
            v_sb = big.tile([128, 8, 256], BF, tag="v_sb")
            for tb in range(8):
                r, half = tb // 2, tb % 2
                pp = ps_big.tile([128, 512], F32, tag="mm")
                for ib in range(8):
                    nc.tensor.matmul(
                        pp[:, 0:256],
                        hflat[:, ib, r, half * 128:(half + 1) * 128],
                        wv[:, ib, :], start=(ib == 0), stop=False)
                nc.tensor.matmul(pp[:, 0:256], ones1, vb_rowb,
                                 start=False, stop=True)
                nc.vector.tensor_copy(v_sb[:, tb, :], pp[:, 0:256])
            if debug and l == 0:
                nc.sync.dma_start(out=dbg["dbg_v"][:, :, :], in_=v_sb)

            # attention, head pairs
            o_all = big.tile([128, 2, S], BF, tag="o_all")
            for hp in range(0 if "attn" in skip else 2):
                h0i, h1i = 2 * hp, 2 * hp + 1   # local head indices
                qofb, kofb = hp, 2 + hp
                for qc in range(4):
                    nk = 2 * qc + 2
                    o_ps = ps_o.tile([128, T], F32, tag="o")
                    rs = ps_rs.tile([1, 2, T], F32, tag="rs")
                    for kblk in range(nk):
                        idx = max(kblk - 2 * qc + 2, 0)
                        s0 = ps_big.tile([128, 512], F32, tag="mm")
                        nc.tensor.matmul(
                            s0[:, 0:T],
                            qk_sb[0:64, kofb, kblk * 128:(kblk + 1) * 128],
                            qk_sb[0:64, qofb, qc * T:(qc + 1) * T],
                            start=True, stop=True)
                        s1 = ps_big.tile([128, 512], F32, tag="mm")
                        nc.tensor.matmul(
                            s1[:, 0:T],
                            qk_sb[64:128, kofb, kblk * 128:(kblk + 1) * 128],
                            qk_sb[64:128, qofb, qc * T:(qc + 1) * T],
                            start=True, stop=True, tile_position=(64, 0))
                        pT = attp.tile([128, 2, T], BF, tag="pT")
                        e0 = attp.tile([128, 2, T], BF, tag="e0")
                        nc.scalar.activation(e0[:, 0, :], s0[:, 0:T], AF.Exp,
                                             scale=float(SCALE))
                        nc.scalar.activation(e0[:, 1, :], s1[:, 0:T], AF.Exp,
                                             scale=float(SCALE))
                        nc.vector.tensor_tensor(
                            pT[:, 0, :], e0[:, 0, :], expb_sb[:, h0i, idx, :],
                            OP.mult)
                        nc.vector.tensor_tensor(
                            pT[:, 1, :], e0[:, 1, :], expb_sb[:, h1i, idx, :],
                            OP.mult)
                        nc.tensor.matmul(rs, ones, pT,
                                         start=(kblk == 0), stop=(kblk == nk - 1))
                        nc.tensor.matmul(
                            o_ps[0:64, :], v_sb[:, kblk, h0i * 64:(h0i + 1) * 64],
                            pT[:, 0, :], start=(kblk == 0), stop=(kblk == nk - 1),
                            skip_group_check=True)
                        nc.tensor.matmul(
                            o_ps[64:128, :], v_sb[:, kblk, h1i * 64:(h1i + 1) * 64],
                            pT[:, 1, :], start=(kblk == 0), stop=(kblk == nk - 1),
                            tile_position=(0, 64), skip_group_check=True)
                    rc = sb.tile([1, 2, T], F32, tag="rc")
                    nc.vector.reciprocal(rc, rs)
                    rcb = sb.tile([1, 2, T], BF, tag="rcb")
                    nc.vector.tensor_copy(rcb, rc)
                    bc = ps_big.tile([128, 512], F32, tag="mm")
                    nc.tensor.matmul(bc[:, 0:T], ind_lo, rcb[:, 0, :],
                                     start=True, stop=False)
                    nc.tensor.matmul(bc[:, 0:T], ind_hi, rcb[:, 1, :],
                                     start=False, stop=True)
                    bcs = sb.tile([128, T], BF, tag="bcs")
                    nc.scalar.copy(bcs, bc[:, 0:T])
                    nc.vector.tensor_tensor(
                        o_all[:, hp, qc * T:(qc + 1) * T], o_ps, bcs, OP.mult)
            if debug and l == 0:
                nc.sync.dma_start(out=dbg["dbg_o"][:, :, :], in_=o_all)

            # out-proj partials (my head features x all tokens) -> ReduceScatter
            wo_sh = big.tile([128, 2, HDIM], BF, tag="wo_sh")
            nc.sync.dma_start(
                out=wo_sh, in_=outw[l].rearrange("(ib p) o -> p ib o", p=128))
            rs_in = dram.tile([R, HDIM, T], BF, tag="rs_in")
            rs_out = dram.tile([HDIM, T], BF, tag="rs_out")
            rs_in_v = rs_in.rearrange("r (ob p) t -> p ob r t", p=128)
            for ob in range(8):
                for tch in range(2):
                    pp = ps_big.tile([128, 512], F32, tag="mm")
                    for ib in range(2):
                        nc.tensor.matmul(
                            pp, wo_sh[:, ib, ob * 128:(ob + 1) * 128],
                            o_all[:, ib, tch * 512:(tch + 1) * 512],
                            start=(ib == 0), stop=(ib == 1))
                    zp = sb.tile([128, 2, T], BF, tag="zp")
                    nc.scalar.copy(zp, pp)
                    nc.sync.dma_start(
                        out=rs_in_v[:, ob, 2 * tch:2 * tch + 2, :], in_=zp)
            if no_coll:
                nc.sync.dma_start(out=rs_out[:, :], in_=rs_in[0])
            else:
                nc.gpsimd.collective_compute(
                    "ReduceScatter", OP.add, replica_groups=GROUPS,
                    ins=[rs_in.opt()], outs=[rs_out.opt()])
            zr = big.tile([128, 8, T], BF, tag="zr")
            nc.sync.dma_start(out=zr,
                              in_=rs_out.rearrange("(ob p) t -> p ob t", p=128))
            outb_sb, _u2 = load_gb(outb[l], outb[l], 8, "outb")
            for ob in range(8):
                nc.vector.scalar_tensor_tensor(
                    z[:, ob, :], zr[:, ob, :], outb_sb[:, ob:ob + 1], z[:, ob, :],
                    OP.add, OP.add)

            # lnF -> hf
            gf, bf_ = load_gb(lnF_g[l], lnF_b[l], 8, "lnF")
            hf = big.tile([128, 8, T], BF, tag="hx")
            _layernorm(nc, sb, lnp, ps_big, ps_rs, z, hf, gf, bf_, 1e-5, 8, csts)
            # fc1 + gelu
            f1b_sb = sb.tile([128, 32], F32, tag="f1b")
            nc.sync.dma_start(out=f1b_sb,
                              in_=fc1b[l].rearrange("(fb p) -> p fb", p=128))
            gact = big.tile([128, 32, T], BF, tag="gact")
            for ob in range(0 if "mlp" in skip else 32):
                wf = wpool.tile([128, 8, 128], BF, tag="wo")
                nc.sync.dma_start(
                    out=wf,
                    in_=fc1w[l][:, ob * 128:(ob + 1) * 128]
                        .rearrange("(ib p) o -> p ib o", p=128))
                pp = ps_big.tile([128, 512], F32, tag="mm")
                for ib in range(8):
                    nc.tensor.matmul(pp[:, 0:T], wf[:, ib, :], hf[:, ib, :],
                                     start=(ib == 0), stop=(ib == 7))
                nc.scalar.activation(gact[:, ob, :], pp[:, 0:T], gelu_fn,
                                     bias=f1b_sb[:, ob:ob + 1])
            # fc2 += z
            f2b_sb, _u3 = load_gb(fc2b[l], fc2b[l], 8, "f2b")
            for ob in range(0 if "mlp" in skip else 8):
                wf2 = wpool2.tile([128, 32, 128], BF, tag="wf2")
                nc.sync.dma_start(
                    out=wf2,
                    in_=fc2w[l][:, ob * 128:(ob + 1) * 128]
                        .rearrange("(ib p) o -> p ib o", p=128))
                pp = ps_big.tile([128, 512], F32, tag="mm")
                for ib in range(32):
                    nc.tensor.matmul(pp[:, 0:T], wf2[:, ib, :], gact[:, ib, :],
                                     start=(ib == 0), stop=(ib == 31))
                nc.vector.scalar_tensor_tensor(
                    z[:, ob, :], pp[:, 0:T], f2b_sb[:, ob:ob + 1], z[:, ob, :],
                    OP.add, OP.add)
            if debug and l == 0:
                nc.sync.dma_start(out=dbg["dbg_z1"][:, :, :], in_=z)

        # ---- final LN + head
        gg, bb = load_gb(fin_g, fin_b, 8, "fin")
        hfin = big.tile([128, 8, T], BF, tag="hx")
        _layernorm(nc, sb, lnp, ps_big, ps_rs, z, hfin, gg, bb, 1e-5, 8, csts)
        hw_sb = big.tile([128, 8, NCLS], BF, tag="hw")
        nc.sync.dma_start(out=hw_sb,
                          in_=headw.rearrange("(ib p) o -> p ib o", p=128))
        hb_sb = sb.tile([NCLS, 1], F32, tag="hb")
        nc.sync.dma_start(out=hb_sb, in_=headb[:, None])
        lp = ps_big.tile([128, 512], F32, tag="mm")
        for ib in range(8):
            nc.tensor.matmul(lp[0:NCLS, 0:T], hw_sb[:, ib, :], hfin[:, ib, :],
                             start=(ib == 0), stop=(ib == 7))
        logit_fm = big.tile([NCLS, T], F32, tag="logit")
        nc.scalar.activation(logit_fm, lp[0:NCLS, 0:T], AF.Identity,
                             bias=hb_sb[:, 0:1])
        # transpose to token-major and store
        osb = big.tile([128, 2, NCLS], F32, tag="osb")
        for ch in range(2):
            tp = ps_big.tile([128, 512], F32, tag="mm")
            nc.tensor.transpose(tp[:, 0:NCLS],
                                logit_fm[:, ch * 128:(ch + 1) * 128],
                                ident[0:NCLS, 0:NCLS])
            nc.vector.tensor_copy(osb[:, ch, :], tp[:, 0:NCLS])
        nc.sync.dma_start(out=out.rearrange("(c p) n -> p c n", p=128), in_=osb)

        for p in reversed(pools):
            p.__exit__(None, None, None)
    nc.compile()
    return nc


def prep_inputs(inputs, nlayers=NLAYERS):
    """Host-side: shard + transpose + cast. Returns in_maps (list of 8 dicts)."""
    bf = ml_dtypes.bfloat16
    f32 = np.float32

    x = np.asarray(inputs["x"], f32)            # (2, 5120, 256)
    qkv_w = np.asarray(inputs["qkv_w"], f32)    # (L, 3072, 1024)
    rel = np.asarray(inputs["rel_table"], f32)  # (16, 257)

    shared = {
        "pe_wT": np.ascontiguousarray(np.asarray(inputs["pe_w"], f32).T).astype(bf),
        "pe_b": np.asarray(inputs["pe_b"], f32),
        "pe_ln1_g": np.asarray(inputs["pe_ln1_g"], f32),
        "pe_ln1_b": np.asarray(inputs["pe_ln1_b"], f32),
        "pe_ln2_g": np.asarray(inputs["pe_ln2_g"], f32),
        "pe_ln2_b": np.asarray(inputs["pe_ln2_b"], f32),
        "lnA_g": np.asarray(inputs["lnA_g"], f32)[:nlayers],
        "lnA_b": np.asarray(inputs["lnA_b"], f32)[:nlayers],
        "lnF_g": np.asarray(inputs["lnF_g"], f32)[:nlayers],
        "lnF_b": np.asarray(inputs["lnF_b"], f32)[:nlayers],
        "outb": np.asarray(inputs["out_b"], f32)[:nlayers],
        "fc1w": np.ascontiguousarray(
            np.asarray(inputs["fc1_w"], f32)[:nlayers].transpose(0, 2, 1)).astype(bf),
        "fc1b": np.asarray(inputs["fc1_b"], f32)[:nlayers],
        "fc2w": np.ascontiguousarray(
            np.asarray(inputs["fc2_w"], f32)[:nlayers].transpose(0, 2, 1)).astype(bf),
        "fc2b": np.asarray(inputs["fc2_b"], f32)[:nlayers],
        "fin_g": np.asarray(inputs["final_g"], f32),
        "fin_b": np.asarray(inputs["final_b"], f32),
        "headw": np.ascontiguousarray(np.asarray(inputs["head_w"], f32).T).astype(bf),
        "headb": np.asarray(inputs["head_b"], f32),
    }

    # per-position (p in 0..3) sharded tensors
    per_pos = []
    for p in range(R):
        heads = range(HL * p, HL * p + HL)
        qrows = np.concatenate([np.arange(h * 64, h * 64 + 64) for h in heads])
        krows = HDIM + qrows
        vrows = 2 * HDIM + qrows
        sel_qk = np.concatenate([qrows, krows])
        qkvw_p = np.ascontiguousarray(
            qkv_w[:nlayers, sel_qk, :].transpose(0, 2, 1)).astype(bf)
        vw_p = np.ascontiguousarray(
            qkv_w[:nlayers, vrows, :].transpose(0, 2, 1)).astype(bf)
        qkv_b = np.asarray(inputs["qkv_b"], f32)
        qkvb_p = qkv_b[:nlayers][:, sel_qk].copy()
        vb_p = qkv_b[:nlayers][:, vrows].copy()
        # expb tiles: idx 0 = const (d<=-256); idx 1..3: d = -128, 0, 128
        expb_p = np.zeros((HL, 4, 128, T), np.float32)
        kk = np.arange(128)[:, None]
        qq = np.arange(T)[None, :]
        for hh, h in enumerate(heads):
            expb_p[hh, 0] = np.exp(rel[h, 0])
            for i, d in enumerate([-128, 0, 128]):
                diff = d + kk - qq
                val = np.exp(rel[h, np.clip(diff, -MAX_REL, MAX_REL) + MAX_REL])
                val[diff > 0] = 0.0
                expb_p[hh, 1 + i] = val
        out_w = np.asarray(inputs["out_w"], f32)
        outw_p = np.ascontiguousarray(
            out_w[:nlayers].transpose(0, 2, 1)[:, qrows, :]).astype(bf)
        per_pos.append({
            "qkvw": qkvw_p, "vw": vw_p, "qkvb": qkvb_p, "vb": vb_p,
            "expb": expb_p.astype(bf), "outw": outw_p,
        })

    in_maps = []
    for c in range(N_CORES):
        b, p = c // R, c % R
        xs = x[b, p * T * TIN:(p + 1) * T * TIN, :]        # (1280, 256)
        xTc = np.ascontiguousarray(
            xs.reshape(T, IN_DIM).T)                        # (1280, 256)
        m = {"xT": xTc.astype(f32)}
        m.update(per_pos[p])
        m.update(shared)
        in_maps.append(m)
    return in_maps


_BUILD_CACHE = {}


def _assemble(results):
    B = 2
    out = np.zeros((B, S, NCLS), np.float32)
    for c in range(N_CORES):
        b, p = c // R, c % R
        out[b, p * T:(p + 1) * T, :] = results[c]["out"]
    return out


def kernel(**inputs) -> np.ndarray:
    in_maps = prep_inputs(inputs)
    key = NLAYERS
    if key not in _BUILD_CACHE:
        _BUILD_CACHE[key] = build(NLAYERS)
    nc = _BUILD_CACHE[key]
    res = run_bass_kernel_spmd(nc, in_maps, core_ids=list(range(N_CORES)))
    return _assemble(res.results)


class Executor:
    """jax.jit shard_map executor with device-resident inputs (for timing)."""

    def __init__(self, nc, in_maps):
        import jax
        from jax.sharding import Mesh, NamedSharding, PartitionSpec
        from jax.experimental.shard_map import shard_map
        import concourse.mybir as mybir
        from concourse import bass2jax
        self.jax = jax
        bass2jax.install_neuronx_cc_hook()
        pname = nc.partition_id_tensor.name if nc.partition_id_tensor else None
        in_names, out_names, out_avals, zero_outs = [], [], [], []
        for alloc in nc.m.functions[0].allocations:
            if not isinstance(alloc, mybir.MemoryLocationSet):
                continue
            name = alloc.memorylocations[0].name
            if alloc.kind == "ExternalInput":
                if name != pname:
                    in_names.append(name)
            elif alloc.kind == "ExternalOutput":
                out_names.append(name)
                shape = tuple(alloc.tensor_shape)
                dtype = mybir.dt.np(alloc.dtype)
                out_avals.append(jax.core.ShapedArray(shape, dtype))
                zero_outs.append(np.zeros(shape, dtype))
        n_params = len(in_names)
        n_outs = len(out_avals)
        names_all = list(in_names) + out_names + ([pname] if pname else [])

        def _body(*args):
            operands = list(args)
            if pname is not None:
                operands.append(bass2jax.partition_id_tensor())
            return tuple(bass2jax._bass_exec_p.bind(
                *operands, out_avals=tuple(out_avals), in_names=tuple(names_all),
                out_names=tuple(out_names), lowering_input_output_aliases=(),
                sim_require_finite=True, sim_require_nnan=True, nc=nc))

        devices = jax.devices()[:N_CORES]
        mesh = Mesh(np.asarray(devices), ("core",))
        spec = PartitionSpec("core")
        self.sharded = jax.jit(
            shard_map(_body, mesh=mesh, in_specs=(spec,) * (n_params + n_outs),
                      out_specs=(spec,) * n_outs, check_rep=False),
            donate_argnums=tuple(range(n_params, n_params + n_outs)),
            keep_unused=True)
        per_core = [[np.asarray(m[nm]) for nm in in_names] for m in in_maps]
        sh = NamedSharding(mesh, spec)
        self.dev_in = [jax.device_put(
            np.concatenate([per_core[c][i] for c in range(N_CORES)], 0), sh)
            for i in range(n_params)]
        self.zero_outs = zero_outs
        self.sh = sh
        self.out_names = out_names

    def _zeros(self):
        return [self.jax.device_put(
            np.zeros((N_CORES * z.shape[0], *z.shape[1:]), z.dtype), self.sh)
            for z in self.zero_outs]

    def run(self):
        out = self.sharded(*self.dev_in, *self._zeros())
        self.jax.block_until_ready(out)
        return out

    def time(self, iters=5):
        import time as _time
        self.run()
        self.run()
        best = float("inf")
        for _ in range(iters):
            zo = self._zeros()
            self.jax.block_until_ready(zo)
            t0 = _time.perf_counter()
            out = self.sharded(*self.dev_in, *zo)
            self.jax.block_until_ready(out)
            best = min(best, _time.perf_counter() - t0)
        return best, out
